# revision 11
# baseline (speedup 1.0000x reference)
"""DecoupledCrossAttention Trainium2 kernel (8 NeuronCores, Bass/Tile).

Reference computation (per batch b of 4, DIM=512, 8 heads x 64):
    q = heads(x @ Wq.T + bq)
    x_audio  = attn(q, audio_context;  Wka, bka, Wva, bva)   # m=2048
    x_singer = attn(q, singer_context; Wks, bks, Wvs, bvs)   # m=256
    out = (x_audio + x_singer) @ Wp.T + bp

Sharding: 8 cores = 4 batches x 2 head-groups (4 heads = 256 feat each).
Each core computes its batch/head-group attention and a PARTIAL output
projection (its 256-dim slice of the Wp contraction); the host sums the
two partials per batch and adds bp (the "all-reduce").

Per-core dataflow (activations kept transposed [feature, token], fp32
storage, float32r matmuls = full PE rate):
    qT = WqT.T @ xT          kT = WkT.T @ ctxT        vT = WvT.T @ ctxT
    v_nat = PE-transpose(vT), augmented with a ones column per head
    sT[m,n] = kT_h.T @ qT_h  (per head, per 128-row m-tile)
    eT = exp(SCALE * sT)     (scalar engine, straight from PSUM)
    pv[65,n] = [v_h|1].T @ eT  accumulated over m-tiles
               rows 0..63 = unnormalized o^T, row 64 = softmax denom
    z = pv[0:64] * (1/denom broadcast) summed over audio+singer + bias
    out_t = WpT.T @ z        (partial over this core's 256 features)
"""
import numpy as np
from contextlib import ExitStack

import concourse.bass as bass
import concourse.tile as tile
from concourse import bacc, mybir
from concourse import bass_utils
from concourse.masks import make_identity

F32 = mybir.dt.float32
F32R = mybir.dt.float32r
AF = mybir.ActivationFunctionType
OP = mybir.AluOpType

DIM = 512
HEADS_PER_CORE = 4   # head-group size (2 groups of 4 heads)
HS = 256             # feature slice per core (4 heads x 64)
HD = 64              # head dim
N = 2048             # query tokens
MA = 2048            # audio context tokens
MS = 256             # singer context tokens
B = 4
SCALE = float(DIM) ** -0.5


def _build(dbg=False):
    nc = bacc.Bacc("TRN2", target_bir_lowering=False, debug=False,
                   enable_asserts=True, num_devices=8)

    def din(name, shape):
        return nc.dram_tensor(name, shape, F32, kind="ExternalInput").ap()

    xT = din("xT", [DIM, N])
    caT = din("caT", [DIM, MA])
    csT = din("csT", [DIM, MS])
    wqT = din("wqT", [DIM, HS])
    wkaT = din("wkaT", [DIM, HS])
    wvaT = din("wvaT", [DIM, HS])
    wksT = din("wksT", [DIM, HS])
    wvsT = din("wvsT", [DIM, HS])
    wpT = din("wpT", [HS, DIM])
    bq = din("bq", [HS])
    bka = din("bka", [HS])
    bks = din("bks", [HS])
    bvv = din("bvv", [HS])
    out_t = nc.dram_tensor("out_t", [DIM, N], F32, kind="ExternalOutput").ap()
    dbg_aps = {}
    if dbg:
        for nm, shp in [("d_qT", [128, 2, N]), ("d_kaT", [128, 2, MA]),
                        ("d_vaT", [128, 2, MA]),
                        ("d_van", [128, MA // 128, 4, HD + 1]),
                        ("d_zT", [128, 2, N]), ("d_eT0", [128, N]),
                        ("d_pv0", [65, N]), ("d_rb0", [128, N])]:
            dbg_aps[nm] = nc.dram_tensor(nm, shp, F32,
                                         kind="ExternalOutput").ap()

    with tile.TileContext(nc) as tc, ExitStack() as ctx:
        const = ctx.enter_context(tc.tile_pool(name="const", bufs=1))
        ctxp = ctx.enter_context(tc.tile_pool(name="ctxp", bufs=2))
        csp = ctx.enter_context(tc.tile_pool(name="csp", bufs=1))
        actp = ctx.enter_context(tc.tile_pool(name="actp", bufs=1))
        vnp = ctx.enter_context(tc.tile_pool(name="vnp", bufs=1))
        shp = ctx.enter_context(tc.tile_pool(name="shp", bufs=2))

        # --- constants -------------------------------------------------
        ident_f = const.tile([128, 128], F32)
        make_identity(nc, ident_f)
        ident = const.tile([128, 128], F32R)
        nc.vector.tensor_copy(ident[:], ident_f[:])

        def load_bias(ap, name):
            t = const.tile([128, 2, 1], F32, name=name)
            src = ap.rearrange("(mt p one) -> mt p one", p=128, one=1)
            for mt in range(2):
                nc.sync.dma_start(out=t[:, mt, :], in_=src[mt])
            return t

        ones_f = const.tile([128, 4, 1], F32)
        nc.vector.memset(ones_f[:], 1.0)
        ones_r = const.tile([128, 4, 1], F32R)
        nc.vector.tensor_copy(ones_r[:], ones_f[:])

        bq_t = load_bias(bq, "bq_t")
        bka_t = load_bias(bka, "bka_t")
        bks_t = load_bias(bks, "bks_t")
        bvv_t = load_bias(bvv, "bvv_t")

        # --- load + round inputs to fp32r (in-place DMA then round) ----
        _ld = [0]

        def load_round(pool, src_ap, width, tag, nt=4):
            """HBM [nt*128, width] fp32 -> SBUF [128, nt, width] fp32r."""
            dst = pool.tile([128, nt, width], F32R, tag=tag, name=tag)
            s3 = src_ap.rearrange("(ct p) w -> ct p w", p=128)
            for ct in range(nt):
                _ld[0] += 1
                st = shp.tile([128, 2048], F32, tag="buf8",
                              name=f"st{_ld[0]}")
                nc.sync.dma_start(out=st[:, :width], in_=s3[ct])
                nc.vector.tensor_copy(dst[:, ct, :], st[:, :width])
            return dst

        xTr = load_round(ctxp, xT, N, tag="ctxT")
        caTr = load_round(ctxp, caT, MA, tag="ctxT")
        csTr = load_round(csp, csT, MS, tag="csT")
        wpTr = load_round(const, wpT, DIM, tag="wpTr", nt=2)

        # --- phase 1: projections (+ v_nat), own PSUM/weight scope -----
        with ExitStack() as p1:
            wpool = p1.enter_context(tc.tile_pool(name="wpool", bufs=1))
            psA = p1.enter_context(tc.tile_pool(name="psA", bufs=4,
                                                space="PSUM"))
            psB = p1.enter_context(tc.tile_pool(name="psB", bufs=2,
                                                space="PSUM"))

            wqTr = load_round(wpool, wqT, HS, "wqTr")
            wkaTr = load_round(wpool, wkaT, HS, "wkaTr")
            wvaTr = load_round(wpool, wvaT, HS, "wvaTr")
            wksTr = load_round(wpool, wksT, HS, "wksTr")
            wvsTr = load_round(wpool, wvsT, HS, "wvsTr")

            def project(w_t, src, width, out_tag, bias=None):
                """[128, 2, width] fp32r = w_t.T @ src (+bias/partition)."""
                chunk = min(512, width)
                nch = width // chunk
                dst = actp.tile([128, 2, width], F32R, tag=out_tag,
                                name=out_tag)
                for mt in range(2):
                    accs = [psA.tile([128, 512], F32, tag="proj",
                                     name=f"pj_{out_tag}_{mt}_{i}")
                            for i in range(nch)]
                    for ct in range(4):
                        lhs = w_t[:, ct, mt * 128:(mt + 1) * 128]
                        for ni in range(nch):
                            nc.tensor.matmul(
                                accs[ni][:, :chunk],
                                lhs,
                                src[:, ct, ni * chunk:(ni + 1) * chunk],
                                start=(ct == 0), stop=(ct == 3),
                            )
                    for ni in range(nch):
                        d = dst[:, mt, ni * chunk:(ni + 1) * chunk]
                        if bias is not None:
                            nc.vector.tensor_scalar_add(
                                d, accs[ni][:, :chunk], bias[:, mt, :])
                        else:
                            nc.vector.tensor_copy(d, accs[ni][:, :chunk])
                return dst

            qT = project(wqTr, xTr, N, "qT", bias=bq_t)
            kaT = project(wkaTr, caTr, MA, "kaT", bias=bka_t)
            vaT = project(wvaTr, caTr, MA, "vaT")
            ksT = project(wksTr, csTr, MS, "ksT", bias=bks_t)
            vsT = project(wvsTr, csTr, MS, "vsT")
            if dbg:
                nc.sync.dma_start(out=dbg_aps["d_qT"], in_=qT[:].bitcast(F32))
                nc.sync.dma_start(out=dbg_aps["d_kaT"], in_=kaT[:].bitcast(F32))
                nc.sync.dma_start(out=dbg_aps["d_vaT"], in_=vaT[:].bitcast(F32))

            # v natural layout with ones column: [128, mt, 4, 65]
            def v_nat_from(vT_t, m_total, tag):
                mts = m_total // 128
                vn = vnp.tile([128, mts, HEADS_PER_CORE, HD + 1], F32R,
                              tag=tag, name=tag)
                for m_t in range(mts):
                    nc.vector.tensor_copy(vn[:, m_t, :, HD:HD + 1], ones_r[:])
                    pt = psB.tile([128, 2, 128], F32R, tag="tps",
                                  name=f"tp_{tag}_{m_t}")
                    for dt_i in range(2):
                        nc.tensor.transpose(
                            pt[:, dt_i, :],
                            vT_t[:, dt_i, m_t * 128:(m_t + 1) * 128],
                            ident[:])
                    nc.vector.tensor_copy(
                        vn[:, m_t, :, 0:HD],
                        pt[:].rearrange("p a (h2 d) -> p (a h2) d", h2=2))
                return vn

            va_n = v_nat_from(vaT, MA, "va_n")
            if dbg:
                nc.sync.dma_start(out=dbg_aps["d_van"],
                                  in_=va_n[:].bitcast(F32))
            vs_n = v_nat_from(vsT, MS, "vs_n")

        zT = actp.tile([128, 2, N], F32R, tag="vaT", name="zT")  # vaT slot

        # --- phase 2: attention ----------------------------------------
        with ExitStack() as p2:
            psQK = p2.enter_context(tc.tile_pool(name="psQK", bufs=2,
                                                 space="PSUM"))
            psPV = p2.enter_context(tc.tile_pool(name="psPV", bufs=4,
                                                 space="PSUM"))
            rpool = p2.enter_context(tc.tile_pool(name="rpool", bufs=2))

            for h in range(HEADS_PER_CORE):
                prow = (h % 2) * 64
                mt_h = h // 2
                q_h = qT[prow:prow + 64, mt_h, :]

                def attend(kT_t, vn_t, m_total, nm):
                    k_h = kT_t[prow:prow + 64, mt_h, :]
                    mts = m_total // 128
                    pv = [psPV.tile([65, 512], F32, tag="pv",
                                    name=f"pv{h}_{nm}_{i}")
                          for i in range(4)]
                    for m_t in range(mts):
                        eT = shp.tile([128, N], F32R, tag="buf8",
                                      name=f"eT{h}_{nm}_{m_t}")
                        for nc2 in range(2):
                            sA = psQK.tile([128, 1024], F32, tag="sA",
                                           name=f"sA{h}_{nm}_{m_t}_{nc2}")
                            for j in range(2):
                                n0 = nc2 * 1024 + j * 512
                                nc.tensor.matmul(
                                    sA[:, j * 512:(j + 1) * 512],
                                    k_h[:, m_t * 128:(m_t + 1) * 128],
                                    q_h[:, n0:n0 + 512],
                                    start=True, stop=True)
                            nc.scalar.activation(
                                eT[:, nc2 * 1024:(nc2 + 1) * 1024], sA[:],
                                AF.Exp, scale=SCALE)
                        if dbg and h == 0 and nm == "a" and m_t == 0:
                            nc.sync.dma_start(out=dbg_aps["d_eT0"],
                                              in_=eT[:].bitcast(F32))
                        lhs_v = vn_t[:, m_t, h, :]
                        for ni in range(4):
                            nc.tensor.matmul(
                                pv[ni][:],
                                lhs_v,
                                eT[:, ni * 512:(ni + 1) * 512],
                                start=(m_t == 0), stop=(m_t == mts - 1))
                    return pv

                z_h = zT[prow:prow + 64, mt_h, :]

                def normalize(pv, nm):
                    """rb rows <- broadcast of 1/pv[64]; returns rb."""
                    rb = rpool.tile([128, N], F32, tag="rb", name=f"rb_{nm}")
                    for ni in range(4):
                        sl = slice(ni * 512, (ni + 1) * 512)
                        # PSUM denom row (p64) -> SBUF p64 -> DMA to p0
                        nc.vector.tensor_copy(rb[64:65, sl],
                                              pv[ni][64:65, :])
                    nc.sync.dma_start(out=rb[0:1, :], in_=rb[64:65, :])
                    nc.vector.reciprocal(rb[0:1, :], rb[0:1, :])
                    nc.gpsimd.partition_broadcast(rb[:], rb[0:1, :])
                    return rb

                # audio
                pv_a = attend(kaT, va_n, MA, "a")
                if dbg and h == 0:
                    for ni in range(4):
                        dpv = rpool.tile([65, 512], F32, tag="dpv",
                                         name=f"dpv{ni}")
                        nc.vector.tensor_copy(dpv[:], pv_a[ni][:])
                        nc.sync.dma_start(
                            out=dbg_aps["d_pv0"][:, ni * 512:(ni + 1) * 512],
                            in_=dpv[:])
                rb_a = normalize(pv_a, f"a{h}")
                if dbg and h == 0:
                    nc.sync.dma_start(out=dbg_aps["d_rb0"], in_=rb_a[:])
                for ni in range(4):
                    sl = slice(ni * 512, (ni + 1) * 512)
                    nc.vector.tensor_tensor(
                        z_h[:, sl], pv_a[ni][0:64, :],
                        rb_a[prow:prow + 64, sl], op=OP.mult)

                # singer
                pv_s = attend(ksT, vs_n, MS, "s")
                rb_s = normalize(pv_s, f"s{h}")
                tmp = rpool.tile([128, N], F32, tag="rb", name=f"tmp{h}")
                for ni in range(4):
                    sl = slice(ni * 512, (ni + 1) * 512)
                    nc.vector.tensor_tensor(
                        tmp[prow:prow + 64, sl], pv_s[ni][0:64, :],
                        rb_s[prow:prow + 64, sl], op=OP.mult)
                    # z = (tmp + bvv) + z
                    nc.vector.scalar_tensor_tensor(
                        z_h[:, sl], tmp[prow:prow + 64, sl],
                        bvv_t[prow:prow + 64, mt_h, :],
                        z_h[:, sl], op0=OP.add, op1=OP.add)

        if dbg:
            nc.sync.dma_start(out=dbg_aps["d_zT"], in_=zT[:].bitcast(F32))

        # --- phase 3: output projection (partial) ----------------------
        with ExitStack() as p3:
            psO = p3.enter_context(tc.tile_pool(name="psO", bufs=4,
                                                space="PSUM"))
            ostage = p3.enter_context(tc.tile_pool(name="ostage", bufs=3))
            for ot in range(4):
                accs = [psO.tile([128, 512], F32, tag="po",
                                 name=f"po{ot}_{i}") for i in range(4)]
                for ft in range(2):
                    lhs = wpTr[:, ft, ot * 128:(ot + 1) * 128]
                    for ni in range(4):
                        nc.tensor.matmul(accs[ni][:], lhs,
                                         zT[:, ft, ni * 512:(ni + 1) * 512],
                                         start=(ft == 0), stop=(ft == 1))
                for ni in range(4):
                    ob = ostage.tile([128, 512], F32, tag="ob",
                                     name=f"ob{ot}_{ni}")
                    nc.vector.tensor_copy(ob[:], accs[ni][:])
                    nc.sync.dma_start(
                        out=out_t[ot * 128:(ot + 1) * 128,
                                  ni * 512:(ni + 1) * 512],
                        in_=ob[:])

    nc.compile()
    return nc


_CACHE = {}


def _get_nc():
    if "nc" not in _CACHE:
        _CACHE["nc"] = _build()
    return _CACHE["nc"]


def _make_in_maps(inputs):
    x = np.asarray(inputs["x"], np.float32)
    ca = np.asarray(inputs["audio_context"], np.float32)
    cs = np.asarray(inputs["singer_context"], np.float32)
    W = {k: np.asarray(inputs[k], np.float32)
         for k in ("Wq", "Wka", "Wva", "Wks", "Wvs", "Wp")}
    bias = {k: np.asarray(inputs[k], np.float32)
            for k in ("bq", "bka", "bva", "bks", "bvs", "bp")}

    c = np.ascontiguousarray
    in_maps = []
    for core in range(8):
        bi, hg = core // 2, core % 2
        hs = slice(hg * HS, (hg + 1) * HS)
        in_maps.append({
            "xT": c(x[bi].T),
            "caT": c(ca[bi].T),
            "csT": c(cs[bi].T),
            "wqT": c(W["Wq"][hs, :].T),
            "wkaT": c(W["Wka"][hs, :].T),
            "wvaT": c(W["Wva"][hs, :].T),
            "wksT": c(W["Wks"][hs, :].T),
            "wvsT": c(W["Wvs"][hs, :].T),
            "wpT": c(W["Wp"][:, hs].T),
            "bq": c(bias["bq"][hs]),
            "bka": c(bias["bka"][hs]),
            "bks": c(bias["bks"][hs]),
            "bvv": c(bias["bva"][hs] + bias["bvs"][hs]),
        })
    return in_maps


def kernel(**inputs) -> np.ndarray:
    nc = _get_nc()
    in_maps = _make_in_maps(inputs)
    res = bass_utils.run_bass_kernel_spmd(nc, in_maps, core_ids=list(range(8)))
    bp = np.asarray(inputs["bp"], np.float32)
    out = np.empty((B, N, DIM), np.float32)
    for bi in range(B):
        s = res.results[2 * bi]["out_t"] + res.results[2 * bi + 1]["out_t"]
        out[bi] = s.T + bp
    return out
